# revision 49
# baseline (speedup 1.0000x reference)
"""DecayMaskedMultiHeadAttention on 8 trn2 NeuronCores (Bass/Tile SPMD).

Model: B=4, N=1024, DIM=1024, 16 heads x head_dim 64.
  q/k/v = x @ W.T + b ; scores = (q_h k_h^T)/8 * decaymask_h ;
  out = softmax(scores) v_h ; y = concat_h(out) @ wo.T + bo

Sharding (8 cores): 4 head-groups x 2 batch-groups.
  core c: head group g = c // 2 (heads 4g..4g+3), batch group p = c % 2
  (batches 2p, 2p+1). Each core computes a partial y (f16) for its 2
  batches; host sums the 4 partials per batch group and adds the
  closed-form bias terms (bo + bv @ wo.T; attention rows sum to 1 so bv
  passes through).

Schedule notes (TRN2: Pool/GPSIMD cannot touch PSUM, so all PSUM-side
element-wise work lives on DVE + ACT):
  - Attention runs as TWO concurrent (head, batch) streams interleaved
    kc-by-kc with the attn@v matmuls software-pipelined one kc behind
    the scores, so the PE never waits on a single
    score->mask-mul->exp->attn@v chain (PE duty keeps the clock p-state
    at 2.4 GHz).
  - v tiles carry 64 ones-columns per head: the attn@v matmul emits the
    softmax denominator already replicated on partitions 64..127 at no
    extra moving-row cost, so normalization is one
    reciprocal_approx_fast [64,1024] + one tensor_mul [64,1024] per
    stream on DVE - no partition_broadcast, no slow reciprocal.
  - Scores for both q-halves land in one 2-bank [128,1024] PSUM tile:
    mask-mul is a single DVE op and exp a single ACT op per (stream,kc).
  - b1's projections ride as PE fillers inside the first two attention
    pairs; b0's out-projection rides inside b1's attention; only b1's
    out-projection is tail.
  - Batched DMAs (one per weight tensor / x batch / mask head,
    [128,1024] f16 output stores), all issued from SP.
"""

import numpy as np
import ml_dtypes

DIM = 1024
H = 16
HD = 64
B = 4
N = 1024
NCORES = 8
HPC = 4            # heads per core
BPC = 2            # batches per core
NSTACK = 2         # 2-head stacks per core
VBLK = 2 * HD      # v block width per head (64 v cols + 64 ones cols)
VROW = HPC * VBLK  # v columns per 128-token chunk

KC = DIM // 128  # 8 contraction chunks over D
TC = N // 128    # 8 token chunks
QH = N // 512    # 2 q halves (fp32 PSUM bank limit is 512)

_PROGRAM = None
LAST_RESULTS = None  # BassKernelResults from the most recent run (for test.py)


def _build_program():
    import concourse.mybir as mybir
    import concourse.tile as tile
    from concourse import bacc

    f32 = mybir.dt.float32
    f16 = mybir.dt.float16
    AF = mybir.ActivationFunctionType

    nc = bacc.Bacc(
        "TRN2",
        target_bir_lowering=False,
        debug=False,
        num_devices=NCORES,
    )

    xT = nc.dram_tensor("xT", [BPC, DIM, N], f16, kind="ExternalInput").ap()
    maskT = nc.dram_tensor("maskT", [HPC, N, N], f16, kind="ExternalInput").ap()
    wqT = nc.dram_tensor("wqT", [DIM, HPC * HD], f16, kind="ExternalInput").ap()
    wkT = nc.dram_tensor("wkT", [DIM, HPC * HD], f16, kind="ExternalInput").ap()
    wvT = nc.dram_tensor("wvT", [DIM, HPC * HD], f16, kind="ExternalInput").ap()
    woT = nc.dram_tensor("woT", [HPC * HD, DIM], f16, kind="ExternalInput").ap()
    bqd = nc.dram_tensor("bq", [NSTACK, 128, 1], f32, kind="ExternalInput").ap()
    bkd = nc.dram_tensor("bk", [NSTACK, 128, 1], f32, kind="ExternalInput").ap()
    outp = nc.dram_tensor("outp", [BPC, N, DIM], f16, kind="ExternalOutput").ap()

    with tile.TileContext(nc) as tc:
        with (
            tc.tile_pool(name="w", bufs=1) as wpool,
            tc.tile_pool(name="persist", bufs=1) as persist,
            tc.tile_pool(name="expm", bufs=2) as expm_p,
            tc.tile_pool(name="expo", bufs=5) as expo_p,
            tc.tile_pool(name="ev", bufs=5) as ev_p,
            tc.tile_pool(name="small", bufs=2) as small_p,
            tc.tile_pool(name="psB", bufs=2, space="PSUM") as psB,
            tc.tile_pool(name="psO", bufs=1, space="PSUM") as psO,
        ):
            # ---- persistent SBUF tiles -------------------------------------
            wq_t = wpool.tile([128, KC * 256], f16, tag="wq", name="wq")
            wk_t = wpool.tile([128, KC * 256], f16, tag="wk", name="wk")
            wv_t = wpool.tile([128, KC * 256], f16, tag="wv", name="wv")
            wo_t = wpool.tile([128, NSTACK * DIM], f16, tag="wo", name="wo")
            bq_t = [wpool.tile([128, 1], f32, tag=f"bq{s}", name=f"bq{s}") for s in range(NSTACK)]
            bk_t = [wpool.tile([128, 1], f32, tag=f"bk{s}", name=f"bk{s}") for s in range(NSTACK)]

            xts = {b: persist.tile([128, KC * N], f16, tag=f"x{b}", name=f"x{b}")
                   for b in range(BPC)}
            mk = {h: persist.tile([128, TC * N], f16, tag=f"mk{h}", name=f"mk{h}")
                  for h in range(HPC)}

            qT = {}
            kT = {}
            ao = {}
            for b in range(BPC):
                for s in range(NSTACK):
                    qT[(b, s)] = persist.tile([128, N], f16, tag=f"qT{b}{s}", name=f"qT{b}{s}")
                    kT[(b, s)] = persist.tile([128, N], f16, tag=f"kT{b}{s}", name=f"kT{b}{s}")
                    ao[(b, s)] = persist.tile([128, N], f16, tag=f"ao{b}{s}", name=f"ao{b}{s}")
            vt = {b: persist.tile([128, TC * VROW], f16, tag=f"v{b}", name=f"v{b}")
                  for b in range(BPC)}
            # whole-tile ones fill (on Pool; SBUF-only engine); v evacs
            # overwrite columns 0..63 of each head block, leaving columns
            # 64..127 = 1.0 (replicated-denominator columns)
            nc.gpsimd.memset(vt[0][:], 1.0)
            nc.gpsimd.memset(vt[1][:], 1.0)

            # ---- input DMAs (all on SP queue, in consumption order) --------
            def load_inputs():
                half = (KC // 2) * 128

                def chunked(dst, src, w):
                    nc.sync.dma_start(
                        dst.rearrange("p (c n) -> p c n", n=w),
                        src.rearrange("(c p) n -> p c n", p=128),
                    )

                q4 = (KC // 4) * 128
                chunked(wq_t[:], wqT, 256)
                for qq in range(4):
                    chunked(xts[0][:, qq * 2 * N:(qq + 1) * 2 * N],
                            xT[0, qq * q4:(qq + 1) * q4, :], N)
                    if qq == 0:
                        for s in range(NSTACK):
                            nc.sync.dma_start(bq_t[s][:], bqd[s])
                            nc.sync.dma_start(bk_t[s][:], bkd[s])
                    if qq == 1:
                        chunked(wk_t[:], wkT, 256)
                chunked(wv_t[:], wvT, 256)
                chunked(mk[0][:], maskT[0], N)
                chunked(xts[1][:, 0:KC // 2 * N], xT[1, 0:half, :], N)
                chunked(mk[1][:], maskT[1], N)
                chunked(xts[1][:, KC // 2 * N:], xT[1, half:, :], N)
                chunked(wo_t[:], woT, DIM)
                chunked(mk[2][:], maskT[2], N)
                chunked(mk[3][:], maskT[3], N)

            # ---- projections (PSUM tiles from the shared 2-bank ring) ------
            def qk_group(b, which, s, qh):
                wt, bt, dst = ((wq_t, bq_t, qT) if which == "q"
                               else (wk_t, bk_t, kT))
                ps = psB.tile([128, N], f32, tag="big", name=f"pj{which}{b}{s}{qh}")
                for kc in range(KC):
                    nc.tensor.matmul(
                        ps[:, 0:512],
                        lhsT=wt[:, kc * 256 + s * 128:kc * 256 + (s + 1) * 128],
                        rhs=xts[b][:, kc * N + qh * 512:kc * N + (qh + 1) * 512],
                        start=(kc == 0),
                        stop=(kc == KC - 1),
                    )
                nc.scalar.activation(
                    dst[(b, s)][:, qh * 512:(qh + 1) * 512],
                    ps[:, 0:512],
                    AF.Identity,
                    bias=bt[s][:],
                    scale=1.0,
                )

            def v_group(b, tci):
                ps = psB.tile([128, N], f32, tag="big", name=f"pjv{b}{tci}")
                for kc in range(KC):
                    nc.tensor.matmul(
                        ps[:, 0:HPC * HD],
                        lhsT=xts[b][:, kc * N + tci * 128:kc * N + (tci + 1) * 128],
                        rhs=wv_t[:, kc * 256:(kc + 1) * 256],
                        start=(kc == 0),
                        stop=(kc == KC - 1),
                    )
                dst = vt[b][:, tci * VROW:(tci + 1) * VROW].rearrange(
                    "p (h e) -> p h e", e=VBLK
                )[:, :, 0:HD]
                nc.scalar.activation(
                    dst, ps[:, 0:HPC * HD].rearrange("p (h e) -> p h e", e=HD),
                    AF.Copy,
                )

            def qk_groups(b, stacks=range(NSTACK)):
                return [lambda b=b, w=which, s=s, qh=qh: qk_group(b, w, s, qh)
                        for s in stacks
                        for which in ("q", "k")
                        for qh in range(QH)]

            def v_groups(b):
                return [lambda b=b, tci=tci: v_group(b, tci)
                        for tci in range(TC)]

            # ---- attention stream machinery --------------------------------
            def sc_mm(st, kc):
                """both q-half score matmuls into one 2-bank PSUM tile"""
                h, b = st
                s, hh = h // 2, h % 2
                sc = psB.tile([128, N], f32, tag="big", name=f"sc{h}{b}{kc}")
                for qh in range(QH):
                    nc.tensor.matmul(
                        sc[:, qh * 512:(qh + 1) * 512],
                        lhsT=kT[(b, s)][hh * HD:(hh + 1) * HD,
                                        kc * 128:(kc + 1) * 128],
                        rhs=qT[(b, s)][hh * HD:(hh + 1) * HD,
                                       qh * 512:(qh + 1) * 512],
                        start=True,
                        stop=True,
                    )
                return sc

            def slot_mul_exp(A, Bst, kc, scA, scB, pool_path=False):
                """mask-muls for both streams + one exp. Normally the muls
                run on DVE straight from PSUM; on pool_path slots the scores
                are ACT-copied to f16 SBUF and the muls run on the
                otherwise-idle Pool engine, shaving the DVE total."""
                em = expm_p.tile([128, 2 * N], f16, tag="expm", name=f"em{kc}{A[0]}{A[1]}")
                if pool_path:
                    sm = scm_p.tile([128, 2 * N], f16, tag="scm",
                                    name=f"sm{kc}{A[0]}{A[1]}")
                    nc.scalar.activation(sm[:, 0:N], scA[:], AF.Copy)
                    nc.scalar.activation(sm[:, N:2 * N], scB[:], AF.Copy)
                    nc.gpsimd.tensor_mul(
                        em[:, 0:N], sm[:, 0:N],
                        mk[A[0]][:, kc * N:(kc + 1) * N])
                    nc.gpsimd.tensor_mul(
                        em[:, N:2 * N], sm[:, N:2 * N],
                        mk[Bst[0]][:, kc * N:(kc + 1) * N])
                else:
                    nc.vector.tensor_mul(
                        em[:, 0:N], scA[:], mk[A[0]][:, kc * N:(kc + 1) * N])
                    nc.vector.tensor_mul(
                        em[:, N:2 * N], scB[:],
                        mk[Bst[0]][:, kc * N:(kc + 1) * N])
                eo = expo_p.tile([128, 2 * N], f16, tag="expo", name=f"eo{kc}{A[0]}{A[1]}")
                nc.scalar.activation(eo[:], em[:], AF.Exp)
                return eo

            def av_mm(st, kc, eo, eoff, op):
                h, b = st
                vblk = vt[b][:, kc * VROW + h * VBLK:kc * VROW + (h + 1) * VBLK]
                for qh in range(QH):
                    nc.tensor.matmul(
                        op[:, qh * 512:(qh + 1) * 512],
                        lhsT=vblk,
                        rhs=eo[:, eoff + qh * 512:eoff + (qh + 1) * 512],
                        start=(kc == 0),
                        stop=(kc == TC - 1),
                    )

            _dn_tiles = {}
            _bcs = {}

            def _dn(pair_key, half):
                """per-pair [128,N] landing tile for both streams' replicated
                denominators (stream A rows 0:64, stream B rows 64:128),
                ACT-evacuated ahead of the combined DVE reciprocal."""
                t = _dn_tiles.get(pair_key)
                if t is None:
                    t = small_p.tile([128, N], f32, tag="dn", name=f"dn{pair_key}")
                    _dn_tiles[pair_key] = t
                return t[half * HD:(half + 1) * HD, :]

            def pair_recip(pair_key):
                """one reciprocal covers both streams (cost is free-size
                based, so [128,N] costs the same as [64,N])."""
                dn = _dn_tiles.pop(pair_key)
                bcs = small_p.tile([128, N], f32, tag="bcs", name=f"bcs{pair_key}")
                nc.vector.reciprocal_approx_fast(bcs[:], dn[:])
                return bcs

            def norm_stream(st, op, bcs, half):
                h, b = st
                s, hh = h // 2, h % 2
                nc.vector.tensor_mul(
                    ao[(b, s)][hh * HD:(hh + 1) * HD, :],
                    op[0:HD, :],
                    bcs[half * HD:(half + 1) * HD, :],
                )

            # ---- out-projection (merged dh; one evac + one store) ----------
            def outproj_group(b, tci, pool="psB", tag="big", dve_evac=False):
                src_pool = psB if pool == "psB" else psO
                po = src_pool.tile([128, N], f32, tag=tag, name=f"po{b}{tci}")
                for dh in range(QH):
                    for s in range(NSTACK):
                        nc.tensor.matmul(
                            po[:, dh * 512:(dh + 1) * 512],
                            lhsT=ao[(b, s)][:, tci * 128:(tci + 1) * 128],
                            rhs=wo_t[:, s * DIM + dh * 512:s * DIM + (dh + 1) * 512],
                            start=(s == 0),
                            stop=(s == NSTACK - 1),
                        )
                ot = ev_p.tile([128, N], f16, tag="ot", name=f"ot{b}{tci}")
                if dve_evac:
                    nc.vector.tensor_copy(ot[:], po[:])
                else:
                    nc.scalar.activation(ot[:], po[:], AF.Copy)
                rows = outp[b, tci * 128:(tci + 1) * 128, :]
                if b == 1 and tci >= TC - 2:
                    nc.gpsimd.dma_start(rows[:, 0:512], ot[:, 0:512])
                    nc.sync.dma_start(rows[:, 512:1024], ot[:, 512:1024])
                else:
                    deng = nc.gpsimd if tci % 2 == 0 else nc.sync
                    deng.dma_start(rows, ot[:])

            # ================= emission =====================================
            load_inputs()

            # all of b0's projections up front (dense PE region while the
            # mask/x1 DMAs stream in); b1's projections + b0's
            # out-projection ride as per-slot PE fillers inside the pairs
            for g in qk_groups(0):
                g()
            for g in v_groups(0):
                g()

            pairs = [((0, 0), (1, 0)), ((2, 0), (3, 0)),
                     ((0, 1), (1, 1)), ((2, 1), (3, 1))]
            fill = {0: qk_groups(1), 1: v_groups(1)}

            prev_ctx = None  # (A, B, opt, eos) of the previous pair

            def boundary(kc, prev_ctx, opt_cur, eos_cur, A, Bst):
                """pair-boundary actions for slot kc: drain the previous
                pair's pipelines (A one slot ahead of B) and start this
                pair's avs as psO slots free up. Returns deferred ACT work
                (dn evacs) to emit after the slot's exp so it cannot delay
                the exp chain in the ACT queue."""
                pA, pB, popt, peos = prev_ctx if prev_ctx else (None,) * 4
                pk = (pA, pB)
                post = []
                if kc == 0 and pA:
                    av_mm(pA, TC - 2, peos[TC - 2], 0, popt[0])
                    av_mm(pB, TC - 3, peos[TC - 3], N, popt[1])
                elif kc == 1 and pA:
                    av_mm(pA, TC - 1, peos[TC - 1], 0, popt[0])
                    av_mm(pB, TC - 2, peos[TC - 2], N, popt[1])
                    post.append(lambda: nc.scalar.activation(
                        _dn(pk, 0), popt[0][HD:2 * HD, :], AF.Copy))
                elif kc == 2:
                    if not pA:
                        av_mm(A, 0, eos_cur[0], 0, opt_cur[0])
                    if pA:
                        av_mm(pB, TC - 1, peos[TC - 1], N, popt[1])

                        def fin_a():
                            nc.scalar.activation(
                                _dn(pk, 1), popt[1][HD:2 * HD, :], AF.Copy)
                            bcs = pair_recip(pk)
                            _bcs[pk] = bcs
                            norm_stream(pA, popt[0], bcs, 0)
                        post.append(fin_a)
                elif kc == 3:
                    if pA:
                        norm_stream(pB, popt[1], _bcs.pop(pk), 1)
                        av_mm(A, 0, eos_cur[0], 0, opt_cur[0])
                    av_mm(A, 1, eos_cur[1], 0, opt_cur[0])
                    av_mm(Bst, 0, eos_cur[0], N, opt_cur[1])
                elif kc >= 4:
                    av_mm(A, kc - 2, eos_cur[kc - 2], 0, opt_cur[0])
                    av_mm(Bst, kc - 3, eos_cur[kc - 3], N, opt_cur[1])
                return post

            for pi, (A, Bst) in enumerate(pairs):
                opt = {
                    slot: psO.tile([128, N], f32, tag=f"op{slot}",
                                   name=f"op{st[0]}{st[1]}")
                    for slot, st in ((0, A), (1, Bst))
                }
                eos = {}
                for kc in range(TC):
                    fl = fill.get(pi)
                    # pair-2 pops must strictly trail the deferred pair-1
                    # finishes (finishB lands in slot 3): first pop slot 4
                    pop = (pi < 2 or (pi == 2 and kc in (4, 6))
                           or (pi == 3 and kc in (0, 1, 2)))
                    if fl and pop:
                        fl.pop(0)()
                    scA = sc_mm(A, kc)
                    scB = sc_mm(Bst, kc)
                    post = boundary(kc, prev_ctx, opt, eos, A, Bst)
                    eos[kc] = slot_mul_exp(A, Bst, kc, scA, scB)
                    for fn in post:
                        fn()
                prev_ctx = (A, Bst, opt, eos)
                if pi == 1:
                    # shared list, popped strictly after the deferred pair-1
                    # finishes land (pair 2 slot 4 onward)
                    op0 = [lambda tci=tci: outproj_group(0, tci)
                           for tci in range(TC)]
                    fill[2] = op0
                    fill[3] = op0
            # drain of the last pair, with the leftover b0 out-projection
            # group keeping the PE busy under the finish chain
            pA, pB, popt, peos = prev_ctx
            av_mm(pA, TC - 2, peos[TC - 2], 0, popt[0])
            av_mm(pB, TC - 3, peos[TC - 3], N, popt[1])
            if fill[3]:
                fill[3].pop(0)()
            if fill[3]:
                fill[3].pop(0)()
            if fill[3]:
                fill[3].pop(0)()
            pk = (pA, pB)
            av_mm(pA, TC - 1, peos[TC - 1], 0, popt[0])
            nc.scalar.activation(_dn(pk, 0), popt[0][HD:2 * HD, :], AF.Copy)
            av_mm(pB, TC - 2, peos[TC - 2], N, popt[1])
            av_mm(pB, TC - 1, peos[TC - 1], N, popt[1])
            nc.scalar.activation(_dn(pk, 1), popt[1][HD:2 * HD, :], AF.Copy)
            bcs = pair_recip(pk)
            norm_stream(pA, popt[0], bcs, 0)
            norm_stream(pB, popt[1], bcs, 1)
            tail_pool = [("psB", "big"), ("psO", "op0"), ("psO", "op1"),
                         ("psB", "big")]
            for tci in range(TC):
                pool, tag = tail_pool[tci % 4]
                outproj_group(1, tci, pool=pool, tag=tag,
                              dve_evac=(tci % 2 == 1))

    nc.compile()
    return nc


def _get_program():
    global _PROGRAM
    if _PROGRAM is None:
        _PROGRAM = _build_program()
    return _PROGRAM


def kernel(x, decaymask, wq, bq, wk, bk, wv, bv, wo, bo):
    from concourse.bass_utils import run_bass_kernel_spmd

    global LAST_RESULTS

    x = np.ascontiguousarray(np.asarray(x, dtype=np.float32))
    decaymask = np.ascontiguousarray(np.asarray(decaymask, dtype=np.float32))
    wq = np.asarray(wq, dtype=np.float32)
    bq = np.asarray(bq, dtype=np.float32)
    wk = np.asarray(wk, dtype=np.float32)
    bk = np.asarray(bk, dtype=np.float32)
    wv = np.asarray(wv, dtype=np.float32)
    bv = np.asarray(bv, dtype=np.float32)
    wo = np.asarray(wo, dtype=np.float32)
    bo = np.asarray(bo, dtype=np.float32)

    nc = _get_program()

    in_maps = []
    for c in range(NCORES):
        g, p = c // 2, c % 2
        rows = slice(g * HPC * HD, (g + 1) * HPC * HD)
        xT_c = np.ascontiguousarray(
            x[p * BPC:(p + 1) * BPC].transpose(0, 2, 1)
        ).astype(np.float16)  # [BPC, D, N]
        maskT_c = np.ascontiguousarray(
            decaymask[g * HPC:(g + 1) * HPC].transpose(0, 2, 1)
        ).astype(np.float16)  # [HPC, k, q]
        # fold 1/sqrt(HD) = 1/8 (exact) into wq/bq
        wqT_c = (np.ascontiguousarray(wq[rows, :].T) * np.float32(0.125)).astype(np.float16)
        wkT_c = np.ascontiguousarray(wk[rows, :].T).astype(np.float16)
        wvT_c = np.ascontiguousarray(wv[rows, :].T).astype(np.float16)
        woT_c = np.ascontiguousarray(wo[:, rows].T).astype(np.float16)
        bq_c = (bq[rows] * np.float32(0.125)).reshape(NSTACK, 128, 1)
        bk_c = bk[rows].reshape(NSTACK, 128, 1).copy()
        in_maps.append({
            "xT": xT_c,
            "maskT": maskT_c,
            "wqT": wqT_c,
            "wkT": wkT_c,
            "wvT": wvT_c,
            "woT": woT_c,
            "bq": np.ascontiguousarray(bq_c),
            "bk": bk_c,
        })

    res = run_bass_kernel_spmd(nc, in_maps, list(range(NCORES)))
    LAST_RESULTS = res

    out = np.zeros((B, N, DIM), dtype=np.float32)
    for c in range(NCORES):
        g, p = c // 2, c % 2
        out[p * BPC:(p + 1) * BPC] += res.results[c]["outp"].astype(np.float32)
    out += (bo + bv @ wo.T)[None, None, :]
    return out


# revision 50
# speedup vs baseline: 1.0366x; 1.0366x over previous
"""DecayMaskedMultiHeadAttention on 8 trn2 NeuronCores (Bass/Tile SPMD).

Model: B=4, N=1024, DIM=1024, 16 heads x head_dim 64.
  q/k/v = x @ W.T + b ; scores = (q_h k_h^T)/8 * decaymask_h ;
  out = softmax(scores) v_h ; y = concat_h(out) @ wo.T + bo

Sharding (8 cores): 4 head-groups x 2 batch-groups.
  core c: head group g = c // 2 (heads 4g..4g+3), batch group p = c % 2
  (batches 2p, 2p+1). Each core computes a partial y (f16) for its 2
  batches; host sums the 4 partials per batch group and adds the
  closed-form bias terms (bo + bv @ wo.T; attention rows sum to 1 so bv
  passes through).

Schedule notes (TRN2: Pool/GPSIMD cannot touch PSUM, so all PSUM-side
element-wise work lives on DVE + ACT):
  - Attention runs as TWO concurrent (head, batch) streams interleaved
    kc-by-kc with the attn@v matmuls software-pipelined one kc behind
    the scores, so the PE never waits on a single
    score->mask-mul->exp->attn@v chain (PE duty keeps the clock p-state
    at 2.4 GHz).
  - v tiles carry 64 ones-columns per head: the attn@v matmul emits the
    softmax denominator already replicated on partitions 64..127 at no
    extra moving-row cost, so normalization is one
    reciprocal_approx_fast [64,1024] + one tensor_mul [64,1024] per
    stream on DVE - no partition_broadcast, no slow reciprocal.
  - Scores for both q-halves land in one 2-bank [128,1024] PSUM tile:
    mask-mul is a single DVE op and exp a single ACT op per (stream,kc).
  - b1's projections ride as PE fillers inside the first two attention
    pairs; b0's out-projection rides inside b1's attention; only b1's
    out-projection is tail.
  - Batched DMAs (one per weight tensor / x batch / mask head,
    [128,1024] f16 output stores), all issued from SP.
"""

import numpy as np
import ml_dtypes

DIM = 1024
H = 16
HD = 64
B = 4
N = 1024
NCORES = 8
HPC = 4            # heads per core
BPC = 2            # batches per core
NSTACK = 2         # 2-head stacks per core
VBLK = 2 * HD      # v block width per head (64 v cols + 64 ones cols)
VROW = HPC * VBLK  # v columns per 128-token chunk

KC = DIM // 128  # 8 contraction chunks over D
TC = N // 128    # 8 token chunks
QH = N // 512    # 2 q halves (fp32 PSUM bank limit is 512)

_PROGRAM = None
LAST_RESULTS = None  # BassKernelResults from the most recent run (for test.py)


def _build_program():
    import concourse.mybir as mybir
    import concourse.tile as tile
    from concourse import bacc

    f32 = mybir.dt.float32
    f16 = mybir.dt.float16
    AF = mybir.ActivationFunctionType

    nc = bacc.Bacc(
        "TRN2",
        target_bir_lowering=False,
        debug=False,
        num_devices=NCORES,
    )

    xT = nc.dram_tensor("xT", [BPC, DIM, N], f16, kind="ExternalInput").ap()
    maskT = nc.dram_tensor("maskT", [HPC, N, N], f16, kind="ExternalInput").ap()
    wqT = nc.dram_tensor("wqT", [DIM, HPC * HD], f16, kind="ExternalInput").ap()
    wkT = nc.dram_tensor("wkT", [DIM, HPC * HD], f16, kind="ExternalInput").ap()
    wvT = nc.dram_tensor("wvT", [DIM, HPC * HD], f16, kind="ExternalInput").ap()
    woT = nc.dram_tensor("woT", [HPC * HD, DIM], f16, kind="ExternalInput").ap()
    bqd = nc.dram_tensor("bq", [NSTACK, 128, 1], f32, kind="ExternalInput").ap()
    bkd = nc.dram_tensor("bk", [NSTACK, 128, 1], f32, kind="ExternalInput").ap()
    outp = nc.dram_tensor("outp", [BPC, N, DIM], f16, kind="ExternalOutput").ap()

    with tile.TileContext(nc) as tc:
        with (
            tc.tile_pool(name="w", bufs=1) as wpool,
            tc.tile_pool(name="persist", bufs=1) as persist,
            tc.tile_pool(name="expm", bufs=2) as expm_p,
            tc.tile_pool(name="expo", bufs=5) as expo_p,
            tc.tile_pool(name="ev", bufs=5) as ev_p,
            tc.tile_pool(name="small", bufs=2) as small_p,
            tc.tile_pool(name="psB", bufs=2, space="PSUM") as psB,
            tc.tile_pool(name="psO", bufs=1, space="PSUM") as psO,
        ):
            # ---- persistent SBUF tiles -------------------------------------
            wq_t = wpool.tile([128, KC * 256], f16, tag="wq", name="wq")
            wk_t = wpool.tile([128, KC * 256], f16, tag="wk", name="wk")
            wv_t = wpool.tile([128, KC * 256], f16, tag="wv", name="wv")
            wo_t = wpool.tile([128, NSTACK * DIM], f16, tag="wo", name="wo")
            bq_t = [wpool.tile([128, 1], f32, tag=f"bq{s}", name=f"bq{s}") for s in range(NSTACK)]
            bk_t = [wpool.tile([128, 1], f32, tag=f"bk{s}", name=f"bk{s}") for s in range(NSTACK)]

            xts = {b: persist.tile([128, KC * N], f16, tag=f"x{b}", name=f"x{b}")
                   for b in range(BPC)}
            mk = {h: persist.tile([128, TC * N], f16, tag=f"mk{h}", name=f"mk{h}")
                  for h in range(HPC)}

            qT = {}
            kT = {}
            ao = {}
            for b in range(BPC):
                for s in range(NSTACK):
                    qT[(b, s)] = persist.tile([128, N], f16, tag=f"qT{b}{s}", name=f"qT{b}{s}")
                    kT[(b, s)] = persist.tile([128, N], f16, tag=f"kT{b}{s}", name=f"kT{b}{s}")
                    ao[(b, s)] = persist.tile([128, N], f16, tag=f"ao{b}{s}", name=f"ao{b}{s}")
            vt = {b: persist.tile([128, TC * VROW], f16, tag=f"v{b}", name=f"v{b}")
                  for b in range(BPC)}
            # whole-tile ones fill (on Pool; SBUF-only engine); v evacs
            # overwrite columns 0..63 of each head block, leaving columns
            # 64..127 = 1.0 (replicated-denominator columns)
            nc.gpsimd.memset(vt[0][:], 1.0)
            nc.gpsimd.memset(vt[1][:], 1.0)

            # ---- input DMAs (all on SP queue, in consumption order) --------
            def load_inputs():
                half = (KC // 2) * 128

                def chunked(dst, src, w):
                    nc.sync.dma_start(
                        dst.rearrange("p (c n) -> p c n", n=w),
                        src.rearrange("(c p) n -> p c n", p=128),
                    )

                q4 = (KC // 4) * 128
                chunked(wq_t[:], wqT, 256)
                for qq in range(4):
                    chunked(xts[0][:, qq * 2 * N:(qq + 1) * 2 * N],
                            xT[0, qq * q4:(qq + 1) * q4, :], N)
                    if qq == 0:
                        for s in range(NSTACK):
                            nc.sync.dma_start(bq_t[s][:], bqd[s])
                            nc.sync.dma_start(bk_t[s][:], bkd[s])
                    if qq == 1:
                        chunked(wk_t[:], wkT, 256)
                chunked(wv_t[:], wvT, 256)
                chunked(mk[0][:], maskT[0], N)
                chunked(xts[1][:, 0:KC // 2 * N], xT[1, 0:half, :], N)
                chunked(mk[1][:], maskT[1], N)
                chunked(xts[1][:, KC // 2 * N:], xT[1, half:, :], N)
                chunked(wo_t[:], woT, DIM)
                chunked(mk[2][:], maskT[2], N)
                chunked(mk[3][:], maskT[3], N)

            # ---- projections (PSUM tiles from the shared 2-bank ring) ------
            def qk_group(b, which, s, qh):
                wt, bt, dst = ((wq_t, bq_t, qT) if which == "q"
                               else (wk_t, bk_t, kT))
                ps = psB.tile([128, N], f32, tag="big", name=f"pj{which}{b}{s}{qh}")
                for kc in range(KC):
                    nc.tensor.matmul(
                        ps[:, 0:512],
                        lhsT=wt[:, kc * 256 + s * 128:kc * 256 + (s + 1) * 128],
                        rhs=xts[b][:, kc * N + qh * 512:kc * N + (qh + 1) * 512],
                        start=(kc == 0),
                        stop=(kc == KC - 1),
                    )
                nc.scalar.activation(
                    dst[(b, s)][:, qh * 512:(qh + 1) * 512],
                    ps[:, 0:512],
                    AF.Identity,
                    bias=bt[s][:],
                    scale=1.0,
                )

            def v_group(b, tci):
                ps = psB.tile([128, N], f32, tag="big", name=f"pjv{b}{tci}")
                for kc in range(KC):
                    nc.tensor.matmul(
                        ps[:, 0:HPC * HD],
                        lhsT=xts[b][:, kc * N + tci * 128:kc * N + (tci + 1) * 128],
                        rhs=wv_t[:, kc * 256:(kc + 1) * 256],
                        start=(kc == 0),
                        stop=(kc == KC - 1),
                    )
                dst = vt[b][:, tci * VROW:(tci + 1) * VROW].rearrange(
                    "p (h e) -> p h e", e=VBLK
                )[:, :, 0:HD]
                nc.scalar.activation(
                    dst, ps[:, 0:HPC * HD].rearrange("p (h e) -> p h e", e=HD),
                    AF.Copy,
                )

            def qk_groups(b, stacks=range(NSTACK)):
                return [lambda b=b, w=which, s=s, qh=qh: qk_group(b, w, s, qh)
                        for s in stacks
                        for which in ("q", "k")
                        for qh in range(QH)]

            def v_groups(b):
                return [lambda b=b, tci=tci: v_group(b, tci)
                        for tci in range(TC)]

            # ---- attention stream machinery --------------------------------
            def sc_mm(st, kc):
                """both q-half score matmuls into one 2-bank PSUM tile"""
                h, b = st
                s, hh = h // 2, h % 2
                sc = psB.tile([128, N], f32, tag="big", name=f"sc{h}{b}{kc}")
                for qh in range(QH):
                    nc.tensor.matmul(
                        sc[:, qh * 512:(qh + 1) * 512],
                        lhsT=kT[(b, s)][hh * HD:(hh + 1) * HD,
                                        kc * 128:(kc + 1) * 128],
                        rhs=qT[(b, s)][hh * HD:(hh + 1) * HD,
                                       qh * 512:(qh + 1) * 512],
                        start=True,
                        stop=True,
                    )
                return sc

            def slot_mul_exp(A, Bst, kc, scA, scB, pool_path=False):
                """mask-muls for both streams + one exp. Normally the muls
                run on DVE straight from PSUM; on pool_path slots the scores
                are ACT-copied to f16 SBUF and the muls run on the
                otherwise-idle Pool engine, shaving the DVE total."""
                em = expm_p.tile([128, 2 * N], f16, tag="expm", name=f"em{kc}{A[0]}{A[1]}")
                if pool_path:
                    sm = scm_p.tile([128, 2 * N], f16, tag="scm",
                                    name=f"sm{kc}{A[0]}{A[1]}")
                    nc.scalar.activation(sm[:, 0:N], scA[:], AF.Copy)
                    nc.scalar.activation(sm[:, N:2 * N], scB[:], AF.Copy)
                    nc.gpsimd.tensor_mul(
                        em[:, 0:N], sm[:, 0:N],
                        mk[A[0]][:, kc * N:(kc + 1) * N])
                    nc.gpsimd.tensor_mul(
                        em[:, N:2 * N], sm[:, N:2 * N],
                        mk[Bst[0]][:, kc * N:(kc + 1) * N])
                else:
                    nc.vector.tensor_mul(
                        em[:, 0:N], scA[:], mk[A[0]][:, kc * N:(kc + 1) * N])
                    nc.vector.tensor_mul(
                        em[:, N:2 * N], scB[:],
                        mk[Bst[0]][:, kc * N:(kc + 1) * N])
                eo = expo_p.tile([128, 2 * N], f16, tag="expo", name=f"eo{kc}{A[0]}{A[1]}")
                nc.scalar.activation(eo[:], em[:], AF.Exp)
                return eo

            def av_mm(st, kc, eo, eoff, op):
                h, b = st
                vblk = vt[b][:, kc * VROW + h * VBLK:kc * VROW + (h + 1) * VBLK]
                for qh in range(QH):
                    nc.tensor.matmul(
                        op[:, qh * 512:(qh + 1) * 512],
                        lhsT=vblk,
                        rhs=eo[:, eoff + qh * 512:eoff + (qh + 1) * 512],
                        start=(kc == 0),
                        stop=(kc == TC - 1),
                    )

            _dn_tiles = {}

            def _dn(st):
                """SBUF landing tile for stream st's replicated denominators
                (ACT-evacuated one slot before the DVE finish)."""
                t = small_p.tile([HD, N], f32, tag="dn", name=f"dn{st[0]}{st[1]}")
                _dn_tiles[st] = t
                return t[:]

            def stream_finish(st, op):
                """reciprocal of the (already SBUF-evacuated) denominators +
                normalize into ao; frees op for the next stream on this
                psO slot."""
                h, b = st
                s, hh = h // 2, h % 2
                dn = _dn_tiles.pop(st)
                bcs = small_p.tile([HD, N], f32, tag="bcs", name=f"bcs{h}{b}")
                nc.vector.reciprocal_approx_fast(bcs[:], dn[:])
                nc.vector.tensor_mul(
                    ao[(b, s)][hh * HD:(hh + 1) * HD, :],
                    op[0:HD, :],
                    bcs[:],
                )

            # ---- out-projection (merged dh; one evac + one store) ----------
            def outproj_group(b, tci, pool="psB", tag="big", dve_evac=False):
                src_pool = psB if pool == "psB" else psO
                po = src_pool.tile([128, N], f32, tag=tag, name=f"po{b}{tci}")
                for dh in range(QH):
                    for s in range(NSTACK):
                        nc.tensor.matmul(
                            po[:, dh * 512:(dh + 1) * 512],
                            lhsT=ao[(b, s)][:, tci * 128:(tci + 1) * 128],
                            rhs=wo_t[:, s * DIM + dh * 512:s * DIM + (dh + 1) * 512],
                            start=(s == 0),
                            stop=(s == NSTACK - 1),
                        )
                ot = ev_p.tile([128, N], f16, tag="ot", name=f"ot{b}{tci}")
                if dve_evac:
                    nc.vector.tensor_copy(ot[:], po[:])
                else:
                    nc.scalar.activation(ot[:], po[:], AF.Copy)
                rows = outp[b, tci * 128:(tci + 1) * 128, :]
                if b == 1 and tci >= TC - 2:
                    nc.gpsimd.dma_start(rows[:, 0:512], ot[:, 0:512])
                    nc.sync.dma_start(rows[:, 512:1024], ot[:, 512:1024])
                else:
                    deng = nc.gpsimd if tci % 2 == 0 else nc.sync
                    deng.dma_start(rows, ot[:])

            # ================= emission =====================================
            load_inputs()

            # all of b0's projections up front (dense PE region while the
            # mask/x1 DMAs stream in); b1's projections + b0's
            # out-projection ride as per-slot PE fillers inside the pairs
            for g in qk_groups(0):
                g()
            for g in v_groups(0):
                g()

            pairs = [((0, 0), (1, 0)), ((2, 0), (3, 0)),
                     ((0, 1), (1, 1)), ((2, 1), (3, 1))]
            fill = {0: qk_groups(1), 1: v_groups(1)}

            prev_ctx = None  # (A, B, opt, eos) of the previous pair

            def boundary(kc, prev_ctx, opt_cur, eos_cur, A, Bst):
                """pair-boundary actions for slot kc: drain the previous
                pair's pipelines (A one slot ahead of B) and start this
                pair's avs as psO slots free up. Returns deferred ACT work
                (dn evacs) to emit after the slot's exp so it cannot delay
                the exp chain in the ACT queue."""
                pA, pB, popt, peos = prev_ctx if prev_ctx else (None,) * 4
                post = []
                if kc == 0 and pA:
                    av_mm(pA, TC - 2, peos[TC - 2], 0, popt[0])
                    av_mm(pB, TC - 3, peos[TC - 3], N, popt[1])
                elif kc == 1 and pA:
                    av_mm(pA, TC - 1, peos[TC - 1], 0, popt[0])
                    av_mm(pB, TC - 2, peos[TC - 2], N, popt[1])
                    post.append(lambda: nc.scalar.activation(
                        _dn(pA), popt[0][HD:2 * HD, :], AF.Copy))
                elif kc == 2:
                    if pA:
                        stream_finish(pA, popt[0])
                        av_mm(pB, TC - 1, peos[TC - 1], N, popt[1])
                        post.append(lambda: nc.scalar.activation(
                            _dn(pB), popt[1][HD:2 * HD, :], AF.Copy))
                    av_mm(A, 0, eos_cur[0], 0, opt_cur[0])
                elif kc == 3:
                    if pA:
                        stream_finish(pB, popt[1])
                    av_mm(A, 1, eos_cur[1], 0, opt_cur[0])
                    av_mm(Bst, 0, eos_cur[0], N, opt_cur[1])
                elif kc >= 4:
                    av_mm(A, kc - 2, eos_cur[kc - 2], 0, opt_cur[0])
                    av_mm(Bst, kc - 3, eos_cur[kc - 3], N, opt_cur[1])
                return post

            for pi, (A, Bst) in enumerate(pairs):
                opt = {
                    slot: psO.tile([128, N], f32, tag=f"op{slot}",
                                   name=f"op{st[0]}{st[1]}")
                    for slot, st in ((0, A), (1, Bst))
                }
                eos = {}
                for kc in range(TC):
                    fl = fill.get(pi)
                    # pair-2 pops must strictly trail the deferred pair-1
                    # finishes (finishB lands in slot 3): first pop slot 4
                    pop = (pi < 2 or (pi == 2 and kc in (4, 6))
                           or (pi == 3 and kc in (0, 1, 2)))
                    if fl and pop:
                        fl.pop(0)()
                    scA = sc_mm(A, kc)
                    scB = sc_mm(Bst, kc)
                    post = boundary(kc, prev_ctx, opt, eos, A, Bst)
                    eos[kc] = slot_mul_exp(A, Bst, kc, scA, scB)
                    for fn in post:
                        fn()
                prev_ctx = (A, Bst, opt, eos)
                if pi == 1:
                    # shared list, popped strictly after the deferred pair-1
                    # finishes land (pair 2 slot 4 onward)
                    op0 = [lambda tci=tci: outproj_group(0, tci)
                           for tci in range(TC)]
                    fill[2] = op0
                    fill[3] = op0
            # drain of the last pair, with the leftover b0 out-projection
            # group keeping the PE busy under the finish chain
            pA, pB, popt, peos = prev_ctx
            av_mm(pA, TC - 2, peos[TC - 2], 0, popt[0])
            av_mm(pB, TC - 3, peos[TC - 3], N, popt[1])
            if fill[3]:
                fill[3].pop(0)()
            if fill[3]:
                fill[3].pop(0)()
            if fill[3]:
                fill[3].pop(0)()
            av_mm(pA, TC - 1, peos[TC - 1], 0, popt[0])
            nc.scalar.activation(_dn(pA), popt[0][HD:2 * HD, :], AF.Copy)
            av_mm(pB, TC - 2, peos[TC - 2], N, popt[1])
            stream_finish(pA, popt[0])
            av_mm(pB, TC - 1, peos[TC - 1], N, popt[1])
            nc.scalar.activation(_dn(pB), popt[1][HD:2 * HD, :], AF.Copy)
            stream_finish(pB, popt[1])
            tail_pool = [("psB", "big"), ("psO", "op0"), ("psO", "op1"),
                         ("psB", "big")]
            for tci in range(TC):
                pool, tag = tail_pool[tci % 4]
                outproj_group(1, tci, pool=pool, tag=tag,
                              dve_evac=(tci % 2 == 1))

    nc.compile()
    return nc


def _get_program():
    global _PROGRAM
    if _PROGRAM is None:
        _PROGRAM = _build_program()
    return _PROGRAM


def kernel(x, decaymask, wq, bq, wk, bk, wv, bv, wo, bo):
    from concourse.bass_utils import run_bass_kernel_spmd

    global LAST_RESULTS

    x = np.ascontiguousarray(np.asarray(x, dtype=np.float32))
    decaymask = np.ascontiguousarray(np.asarray(decaymask, dtype=np.float32))
    wq = np.asarray(wq, dtype=np.float32)
    bq = np.asarray(bq, dtype=np.float32)
    wk = np.asarray(wk, dtype=np.float32)
    bk = np.asarray(bk, dtype=np.float32)
    wv = np.asarray(wv, dtype=np.float32)
    bv = np.asarray(bv, dtype=np.float32)
    wo = np.asarray(wo, dtype=np.float32)
    bo = np.asarray(bo, dtype=np.float32)

    nc = _get_program()

    in_maps = []
    for c in range(NCORES):
        g, p = c // 2, c % 2
        rows = slice(g * HPC * HD, (g + 1) * HPC * HD)
        xT_c = np.ascontiguousarray(
            x[p * BPC:(p + 1) * BPC].transpose(0, 2, 1)
        ).astype(np.float16)  # [BPC, D, N]
        maskT_c = np.ascontiguousarray(
            decaymask[g * HPC:(g + 1) * HPC].transpose(0, 2, 1)
        ).astype(np.float16)  # [HPC, k, q]
        # fold 1/sqrt(HD) = 1/8 (exact) into wq/bq
        wqT_c = (np.ascontiguousarray(wq[rows, :].T) * np.float32(0.125)).astype(np.float16)
        wkT_c = np.ascontiguousarray(wk[rows, :].T).astype(np.float16)
        wvT_c = np.ascontiguousarray(wv[rows, :].T).astype(np.float16)
        woT_c = np.ascontiguousarray(wo[:, rows].T).astype(np.float16)
        bq_c = (bq[rows] * np.float32(0.125)).reshape(NSTACK, 128, 1)
        bk_c = bk[rows].reshape(NSTACK, 128, 1).copy()
        in_maps.append({
            "xT": xT_c,
            "maskT": maskT_c,
            "wqT": wqT_c,
            "wkT": wkT_c,
            "wvT": wvT_c,
            "woT": woT_c,
            "bq": np.ascontiguousarray(bq_c),
            "bk": bk_c,
        })

    res = run_bass_kernel_spmd(nc, in_maps, list(range(NCORES)))
    LAST_RESULTS = res

    out = np.zeros((B, N, DIM), dtype=np.float32)
    for c in range(NCORES):
        g, p = c // 2, c % 2
        out[p * BPC:(p + 1) * BPC] += res.results[c]["outp"].astype(np.float32)
    out += (bo + bv @ wo.T)[None, None, :]
    return out
